# revision 5
# baseline (speedup 1.0000x reference)
"""Multi-head attention kernel for 8 Trainium2 NeuronCores.

Problem: B=2, S=2048, E=1024, H=16 heads, d=64 per head.
Sharding: 8 cores = 2 batches x 4 head-groups (4 heads each).
Each core computes a partial output (its heads' contribution through the
row-split of Wo); the host sums the 4 partials per batch and adds bo.

v3 design (ACT-exp is the 147us floor; keep it streaming):
  - Inputs stream on BOTH hardware DGE queues (sync + scalar) interleaved
    per contraction chunk so the m0 projection starts ~5us in.
  - Flash runs per head-PAIR: the two heads of an mc chunk occupy PE rows
    0-63 / 64-127, so their score matmuls are row-tiled (tile_position
    (0,0) / (64,0)) and execute concurrently on the PE sub-arrays.
  - One shared transient-PSUM rotation (tag "sc", 2x[128,1024] = 4 banks)
    carries score tiles AND injected projection/output-proj groups; the
    o2 accumulators (2x[65,1024] = 4 banks) fill the rest of PSUM.
  - V-projection, m1 projection and out-proj(half0) are injected into the
    PE slack of flash blocks 1, 2 and 4; AV matmuls drain with lag >= 1
    behind the exp stream so injections never stall ACT.
  - Per-head softmax denominators round-trip DRAM ([1,1024] -> [128,8]
    reciprocal -> partition-broadcast) on the fast sync queue; o2 is
    freed right after its two eviction copies.
"""

import numpy as np
import ml_dtypes

import concourse.bass as bass
import concourse.mybir as mybir
import concourse.tile as tile
from concourse.bass_utils import run_bass_kernel_spmd

B, S, E, H, D = 2, 2048, 1024, 16, 64
HPC = 4              # heads per core
DH = HPC * D         # 256 head dims per core
NCORES = 8
P = 128

BF16 = mybir.dt.bfloat16
FP32 = mybir.dt.float32
AF = mybir.ActivationFunctionType


def _dedupe_ldweights(nc):
    """Tile lowers each matmul to InstLdweights + InstMatmult. Consecutive
    matmuls sharing the stationary operand reload identical weights; drop a
    LDW when the previous LDW on the PE stream loaded the same AP and the
    duplicate carries no sync side effects."""
    dropped = 0
    for fn in nc.m.functions:
        for bb in fn.blocks:
            last_key = None
            keep = []
            for inst in bb.instructions:
                tn = type(inst).__name__
                if tn == "InstLdweights":
                    si = getattr(inst, "sync_info", None)
                    key = repr(inst.ins)
                    clean = si is None or (not si.on_wait and not si.on_update)
                    if clean and key == last_key:
                        dropped += 1
                        continue
                    last_key = key
                keep.append(inst)
            bb.instructions.clear()
            bb.instructions.extend(keep)
    return dropped


def _split_waits(nc, k=1):
    """Walrus in this toolchain only accepts one sync-wait per instruction.
    Split any instruction carrying more than k waits by prepending NoOps on
    the same engine, each carrying k of the waits."""
    nid = [0]
    for fn in nc.m.functions:
        for bb in fn.blocks:
            new_insts = []
            for inst in bb.instructions:
                si = getattr(inst, "sync_info", None)
                if si is not None and si.on_wait and len(si.on_wait) > k:
                    waits = list(si.on_wait)
                    while len(waits) > k:
                        chunk, waits = waits[:k], waits[k:]
                        nop = mybir.InstNoOp(
                            name=f"I-splitw-{nid[0]}", ins=[], outs=[]
                        )
                        nid[0] += 1
                        nop.engine = inst.engine
                        nop.sync_info = mybir.SyncInfo(
                            on_update=[], on_wait=list(chunk)
                        )
                        new_insts.append(nop)
                    si.on_wait.clear()
                    si.on_wait.extend(waits)
                new_insts.append(inst)
            bb.instructions.clear()
            bb.instructions.extend(new_insts)


def _build_nc():
    nc = bass.Bass("TRN2", target_bir_lowering=False, debug=False,
                   num_devices=NCORES)

    xqT = nc.dram_tensor("xqT", [E, S], BF16, kind="ExternalInput")
    xkT = nc.dram_tensor("xkT", [E, S], BF16, kind="ExternalInput")
    xvT = nc.dram_tensor("xvT", [E, S], BF16, kind="ExternalInput")
    wq = nc.dram_tensor("wq", [E, DH], BF16, kind="ExternalInput")
    wk = nc.dram_tensor("wk", [E, DH], BF16, kind="ExternalInput")
    wv = nc.dram_tensor("wv", [E, DH], BF16, kind="ExternalInput")
    wo = nc.dram_tensor("wo", [DH, E], BF16, kind="ExternalInput")
    bq = nc.dram_tensor("bq", [DH, 1], FP32, kind="ExternalInput")
    bk = nc.dram_tensor("bk", [DH, 1], FP32, kind="ExternalInput")
    bv = nc.dram_tensor("bv", [1, DH], FP32, kind="ExternalInput")
    out = nc.dram_tensor("out", [S, E], mybir.dt.float16,
                         kind="ExternalOutput")

    EC = E // P           # 8 e-chunks
    MC = DH // P          # 2 d-chunks (= head pairs)
    ST = S // P           # 16 sk-chunks
    SCALE = 1.0 / np.sqrt(np.float32(D))

    with tile.TileContext(nc) as tc:
        with (
            tc.tile_pool(name="consts", bufs=1) as consts,
            tc.tile_pool(name="xbig", bufs=24) as xbig,
            tc.tile_pool(name="qkv", bufs=1) as qkv_pool,
            tc.tile_pool(name="at", bufs=10) as at_pool,
            tc.tile_pool(name="norm", bufs=4) as norm_pool,
            tc.tile_pool(name="o2s", bufs=2) as o2s_pool,
            tc.tile_pool(name="rrep", bufs=2) as rrep_pool,
            tc.tile_pool(name="outs", bufs=3) as out_pool,
            tc.tile_pool(name="dscr", bufs=4, space="DRAM") as dram_pool,
            tc.tile_pool(name="sc", bufs=2, space="PSUM") as sc_pool,
            tc.tile_pool(name="o2", bufs=2, space="PSUM") as o2_pool,
        ):
            # ---- ACT exp-table preload while DMAs stream ----
            wrm = norm_pool.tile([P, 8], FP32, tag="wrm", name="wrm")
            nc.gpsimd.memset(wrm[:], 0.0)
            wrm2 = norm_pool.tile([P, 8], FP32, tag="wrm2", name="wrm2")
            nc.scalar.activation(wrm2[:], wrm[:], AF.Exp)

            # ---- input DMA emission: two HWDGE queues in parallel.
            # scalar queue: K-side (wk, xk) then wv, xv evens.
            # sync queue:   Q-side (wq, xq) then biases, xv odds, wo.
            w_sb = {}
            x_sb = {}
            for name in ("wk", "wq", "wv"):
                w_sb[name] = consts.tile([P, EC, DH], BF16, tag=name,
                                         name=f"w_{name}")
                x_sb[name] = [xbig.tile([P, S], BF16, tag="x",
                                        name=f"x_{name}_{c}")
                              for c in range(EC)]
            for c in range(EC):
                nc.scalar.dma_start(w_sb["wk"][:, c, :],
                                    wk[c * P:(c + 1) * P, :])
                nc.scalar.dma_start(x_sb["wk"][c][:, 0:1024],
                                    xkT[c * P:(c + 1) * P, 0:1024])
                nc.sync.dma_start(w_sb["wq"][:, c, :],
                                  wq[c * P:(c + 1) * P, :])
                nc.sync.dma_start(x_sb["wq"][c][:, 0:1024],
                                  xqT[c * P:(c + 1) * P, 0:1024])
            # biases early on sync (needed by first bias-adds)
            bv_rep = consts.tile([P, DH], FP32, tag="bv")
            bq_sb = consts.tile([P, MC], FP32, tag="bq")
            bk_sb = consts.tile([P, MC], FP32, tag="bk")
            for m in range(MC):
                nc.sync.dma_start(bq_sb[:, m:m + 1], bq[m * P:(m + 1) * P, :])
                nc.sync.dma_start(bk_sb[:, m:m + 1], bk[m * P:(m + 1) * P, :])
            nc.sync.dma_start(bv_rep[:], bv.ap().to_broadcast((P, DH)))
            for c in range(EC):
                nc.scalar.dma_start(x_sb["wk"][c][:, 1024:2048],
                                    xkT[c * P:(c + 1) * P, 1024:2048])
                nc.sync.dma_start(x_sb["wq"][c][:, 1024:2048],
                                  xqT[c * P:(c + 1) * P, 1024:2048])
            for c in range(EC):
                nc.scalar.dma_start(w_sb["wv"][:, c, :],
                                    wv[c * P:(c + 1) * P, :])
            # xv split across both queues
            for c in range(EC):
                eng = nc.scalar if c % 2 == 0 else nc.sync
                eng.dma_start(x_sb["wv"][c][:], xvT[c * P:(c + 1) * P, :])
            wo_sb = consts.tile([P, MC, E], BF16, tag="wo")
            for c in range(MC):
                nc.sync.dma_start(wo_sb[:, c, :], wo[c * P:(c + 1) * P, :])

            # ---- persistent SBUF tensors ----
            qT = qkv_pool.tile([P, MC, S], BF16, tag="qT")
            kT = qkv_pool.tile([P, MC, S], BF16, tag="kT")
            v_sb = qkv_pool.tile([P, ST, HPC, D + 1], BF16, tag="v")
            oT = qkv_pool.tile([P, MC, S], BF16, tag="oT")
            # ones column of V_aug (softmax denominator trick), one strided
            # memset for all token tiles
            nc.gpsimd.memset(v_sb[:, :, :, D:D + 1], 1.0)

            # ---- helpers ----
            def proj_qk_group(w_name, dst, b_sb, m, half):
                """One [128,1024] projection psum group: dst[:,m,half]."""
                ps = sc_pool.tile([P, 1024], FP32, tag="sc",
                                  name=f"pb_{w_name}_{m}_{half}")
                xts = x_sb[w_name]
                for c in range(EC):
                    for n in range(2):
                        nc.tensor.matmul(
                            ps[:, n * 512:(n + 1) * 512],
                            w_sb[w_name][:, c, m * P:(m + 1) * P],
                            xts[c][:, half * 1024 + n * 512:
                                   half * 1024 + (n + 1) * 512],
                            start=(c == 0),
                            stop=(c == EC - 1),
                        )
                nc.vector.tensor_scalar_add(
                    dst[:, m, half * 1024:(half + 1) * 1024],
                    ps[:],
                    b_sb[:, m:m + 1],
                )

            v_ready = [-1]       # highest token tile with v_sb built

            def gen_v_proj():
                """V projection, one token tile per chunk (x-stationary)."""
                for t in range(ST):
                    ps = sc_pool.tile([P, 1024], FP32, tag="sc",
                                      name=f"pv{t}")
                    for c in range(EC):
                        nc.tensor.matmul(
                            ps[:, 0:DH],
                            x_sb["wv"][c][:, t * P:(t + 1) * P],
                            w_sb["wv"][:, c, :],
                            start=(c == 0),
                            stop=(c == EC - 1),
                        )
                    nc.vector.tensor_add(
                        v_sb[:, t, :, 0:D],
                        ps[:, 0:DH].rearrange("p (h d) -> p h d", h=HPC),
                        bv_rep[:].rearrange("p (h d) -> p h d", h=HPC),
                    )
                    v_ready[0] = t
                    yield

            def gen_m1_proj():
                for w_name, dst, b_sb, half in (
                    ("wk", kT, bk_sb, 0), ("wk", kT, bk_sb, 1),
                    ("wq", qT, bq_sb, 0), ("wq", qT, bq_sb, 1),
                ):
                    proj_qk_group(w_name, dst, b_sb, 1, half)
                    yield

            def out_proj_mt(mt):
                ps = sc_pool.tile([P, 1024], FP32, tag="sc",
                                  name=f"po{mt}")
                for c in range(MC):
                    for eh in range(2):
                        nc.tensor.matmul(
                            ps[:, eh * 512:(eh + 1) * 512],
                            oT[:, c, mt * P:(mt + 1) * P],
                            wo_sb[:, c, eh * 512:(eh + 1) * 512],
                            start=(c == 0),
                            stop=(c == MC - 1),
                        )
                ot = out_pool.tile([P, E], mybir.dt.float16, tag="ot")
                nc.vector.tensor_copy(ot[:], ps[:])
                nc.sync.dma_start(out[mt * P:(mt + 1) * P, :], ot[:])

            def gen_out_proj(half):
                for mt in range(half * 8, half * 8 + 8):
                    out_proj_mt(mt)
                    yield

            def norm_head(h, half, o2):
                """Evict o2 fast, then softmax-normalize via DRAM reshape
                reciprocal and partition-broadcast, all on sync HWDGE."""
                mc, po = h // 2, (h % 2) * D
                hb = half * 1024
                o2s = o2s_pool.tile([D, 1024], BF16, tag="o2s")
                nc.vector.tensor_copy(o2s[:], o2[0:D, :])
                dsum = norm_pool.tile([1, 1024], FP32, tag="dsum")
                nc.vector.tensor_copy(dsum[:], o2[D:D + 1, :])
                d1 = dram_pool.tile([1, 1024], FP32, tag="d1")
                nc.sync.dma_start(d1[:], dsum[:])
                dsq = norm_pool.tile([P, 8], FP32, tag="dsq")
                nc.sync.dma_start(
                    dsq[:], d1[:].rearrange("o (p f) -> (o p) f", p=P)
                )
                rsq = norm_pool.tile([P, 8], FP32, tag="rsq")
                nc.vector.reciprocal(rsq[:], dsq[:])
                d2 = dram_pool.tile([P, 8], FP32, tag="d2")
                nc.sync.dma_start(d2[:], rsq[:])
                rrep = rrep_pool.tile([D, 1024], FP32, tag="rrep")
                src = d2[:].rearrange("p f -> (p f)")[None, :]
                nc.sync.dma_start(rrep[:], src.to_broadcast((D, 1024)))
                nc.vector.tensor_mul(
                    oT[po:po + D, mc, hb:hb + 1024], o2s[:], rrep[:]
                )

            # ---- injection pump ----
            from collections import deque
            inj = deque()

            def pump(n=1):
                for _ in range(n):
                    if not inj:
                        return
                    g = inj[0]
                    try:
                        next(g)
                    except StopIteration:
                        inj.popleft()

            # ---- one flash block: head pair mc, query half ----
            def flash_block(mc, half, pump_per_step=1, need_v=False):
                he, ho = 2 * mc, 2 * mc + 1
                hb = half * 1024
                o2e = o2_pool.tile([D + 1, 1024], FP32, tag="o2",
                                   name=f"o2e_{mc}_{half}")
                o2o = o2_pool.tile([D + 1, 1024], FP32, tag="o2",
                                   name=f"o2o_{mc}_{half}")
                avq = deque()

                def drain(force=False):
                    while avq:
                        j0, aTe, aTo = avq[0]
                        if need_v and v_ready[0] < j0 and not force:
                            return
                        if not force and len(avq) <= 1:
                            return
                        avq.popleft()
                        for o2t, aT, h in ((o2e, aTe, he), (o2o, aTo, ho)):
                            for n in range(2):
                                nc.tensor.matmul(
                                    o2t[:, n * 512:(n + 1) * 512],
                                    v_sb[:, j0, h, :],
                                    aT[:, n * 512:(n + 1) * 512],
                                    start=(j0 == 0),
                                    stop=(j0 == ST - 1),
                                )

                for j in range(ST):
                    sce = sc_pool.tile([P, 1024], FP32, tag="sc",
                                       name=f"sce_{mc}_{half}_{j}")
                    sco = sc_pool.tile([P, 1024], FP32, tag="sc",
                                       name=f"sco_{mc}_{half}_{j}")
                    for n in range(2):
                        nc.tensor.matmul(
                            sce[:, n * 512:(n + 1) * 512],
                            kT[0:D, mc, j * P:(j + 1) * P],
                            qT[0:D, mc, hb + n * 512:hb + (n + 1) * 512],
                            start=True, stop=True,
                        )
                    for n in range(2):
                        nc.tensor.matmul(
                            sco[:, n * 512:(n + 1) * 512],
                            kT[D:P, mc, j * P:(j + 1) * P],
                            qT[D:P, mc, hb + n * 512:hb + (n + 1) * 512],
                            start=True, stop=True,
                        )
                    aTe = at_pool.tile([P, 1024], BF16, tag="aT",
                                       name=f"aTe_{mc}_{half}_{j}")
                    nc.scalar.activation(aTe[:], sce[:], AF.Exp, scale=SCALE)
                    aTo = at_pool.tile([P, 1024], BF16, tag="aT",
                                       name=f"aTo_{mc}_{half}_{j}")
                    nc.scalar.activation(aTo[:], sco[:], AF.Exp, scale=SCALE)
                    avq.append((j, aTe, aTo))
                    pump(pump_per_step)
                    drain()
                drain(force=True)
                norm_head(he, half, o2e)
                norm_head(ho, half, o2o)

            # ---- m0 projection (pre-flash) ----
            proj_qk_group("wk", kT, bk_sb, 0, 0)
            proj_qk_group("wq", qT, bq_sb, 0, 0)
            proj_qk_group("wk", kT, bk_sb, 0, 1)
            proj_qk_group("wq", qT, bq_sb, 0, 1)

            # ---- flash blocks; injections ride the PE slack ----
            inj.append(gen_v_proj())
            flash_block(0, 0, pump_per_step=2, need_v=True)
            inj.append(gen_m1_proj())
            flash_block(0, 1, pump_per_step=1, need_v=False)
            flash_block(1, 0, pump_per_step=1, need_v=False)
            inj.append(gen_out_proj(0))
            flash_block(1, 1, pump_per_step=1, need_v=False)
            pump(8)  # finish any leftover out-proj(0) tiles
            for g in gen_out_proj(1):
                pass

    _dedupe_ldweights(nc)
    _split_waits(nc)
    return nc


_NC_CACHE = None


def _get_nc():
    global _NC_CACHE
    if _NC_CACHE is None:
        _NC_CACHE = _build_nc()
    return _NC_CACHE


def _pack_inputs(queries, keys, values, Wq, bq, Wk, bk, Wv, bv, Wo):
    bf16 = ml_dtypes.bfloat16
    in_maps = []
    xT = {}
    for b in range(B):
        xT[b] = (
            np.ascontiguousarray(queries[b].T).astype(bf16),
            np.ascontiguousarray(keys[b].T).astype(bf16),
            np.ascontiguousarray(values[b].T).astype(bf16),
        )
    for b in range(B):
        for hg in range(4):
            heads = [4 * hg + i for i in range(HPC)]
            # interleaved head split: head h owns columns d*H + h
            cols = np.array(
                [d * H + h for h in heads for d in range(D)], dtype=np.int64
            )
            in_maps.append({
                "xqT": xT[b][0],
                "xkT": xT[b][1],
                "xvT": xT[b][2],
                "wq": np.ascontiguousarray(Wq[:, cols]).astype(bf16),
                "wk": np.ascontiguousarray(Wk[:, cols]).astype(bf16),
                "wv": np.ascontiguousarray(Wv[:, cols]).astype(bf16),
                "wo": np.ascontiguousarray(
                    Wo[hg * DH:(hg + 1) * DH, :]
                ).astype(bf16),
                "bq": np.ascontiguousarray(
                    bq[cols].astype(np.float32).reshape(DH, 1)
                ),
                "bk": np.ascontiguousarray(
                    bk[cols].astype(np.float32).reshape(DH, 1)
                ),
                "bv": np.ascontiguousarray(
                    bv[cols].astype(np.float32).reshape(1, DH)
                ),
            })
    return in_maps


def kernel(queries, keys, values, mask, Wq, bq, Wk, bk, Wv, bv, Wo, bo,
           **run_kwargs):
    queries = np.asarray(queries, dtype=np.float32)
    keys = np.asarray(keys, dtype=np.float32)
    values = np.asarray(values, dtype=np.float32)
    nc = _get_nc()
    in_maps = _pack_inputs(queries, keys, values, Wq, bq, Wk, bk, Wv, bv, Wo)
    res = run_bass_kernel_spmd(
        nc, in_maps, core_ids=list(range(NCORES)), **run_kwargs
    )
    bo32 = np.asarray(bo, dtype=np.float32)
    full = np.empty((B, S, E), dtype=np.float32)
    for b in range(B):
        acc = res.results[4 * b]["out"].astype(np.float32)
        # partials come back fp16; accumulate in fp32
        for hg in range(1, 4):
            acc = acc + res.results[4 * b + hg]["out"].astype(np.float32)
        full[b] = acc + bo32
    kernel.last_results = res
    return full
